# revision 20
# baseline (speedup 1.0000x reference)
"""Chunked local attention (B=4, S=8192, D=1024, H=16, Dh=64, C=256, W=64)
on 8 Trainium2 NeuronCores.

Sharding: data-parallel over the 128 (batch x chunk) units -> 16 chunks/core.
The host passes each core its x shard pre-transposed in fp16 ([D, 16*256]);
weights are replicated fp16.

All matmuls run single-pass fp16 (1 PE cycle/row, fp32 PSUM accumulate).
End-to-end max-rel error ~5e-4 against the fp32 reference -- well inside
the 2e-2 gate -- at 3x less PE work than the previous bf16 hi/lo 3-pass
scheme (which targeted 2e-5).

The emission is software-pipelined: chunk c's attention phase (scores,
softmax, PV, transpose, Wo) is interleaved instruction-by-instruction with
chunk c+1's Q/K/V projection matmuls. Without this the attention phase's
small matmuls leave enough PE idle gaps that the HAM clock gate re-throttles
the PE to 1.2 GHz (measured: transpose/score matmuls at exactly the cold
rate); the dense interleaved stream keeps the PE at 2.4 GHz throughout.

ACT/DVE instruction pressure is kept low by batching pairs: both heads of a
score head-pair live in one [128,2,C] PSUM bank (one exp + one mask-mul per
pair), Q/K projection PSUM is written two m-tiles per bank (one copy per
pair), and transposes are copied out two d-tiles at a time.

Per-core dataflow:
  qT/kT = Wq^T/Wk^T @ xT     (lhsT=W native, rhs=xT)        [dout, tok]
  v     = xT^T @ Wv          (lhsT=xT, rhs=Wv)              [tok, dout]
  sT_h  = kT_h^T-x-qT_h      (lhsT=kT_h, rhs=qT_h)          [j, i] per head
  p_h   = exp(0.125*sT_h) * bandmask  (fp16)                [j, i]
  oU|den= p_h^T @ [v_h|1]    (lhsT=p_h, rhs=v_aug)          [i, 65] per head
          (ones column on v gives the softmax denominator for free;
           out-of-band p region is zero-filled so all PV matmuls are
           full-partition with simple start/stop accumulation)
  oN    = oU * (1/den)  per-head bcast, fused in PSUM copy
  oT    = PE-transpose(oN) fp32, fp16 conversion in the PSUM copy
  y     = oT^T @ Wo          (lhsT=oT, rhs=Wo)              [tok, d2] fp16
"""

from contextlib import ExitStack

import numpy as np

import concourse.bass as bass
import concourse.mybir as mybir
import concourse.tile as tile
from concourse import bacc
from concourse.bass_utils import run_bass_kernel_spmd
from concourse.masks import make_identity

B, S, D = 4, 8192, 1024
H, DH, C, W = 16, 64, 256, 64
NCORES = 8
NCHUNKS_TOTAL = B * (S // C)      # 128
CPC = NCHUNKS_TOTAL // NCORES     # 16 chunks per core
TPC = CPC * C                     # 4096 tokens per core
F32 = mybir.dt.float32
F16 = mybir.dt.float16
KT = D // 128                     # 8 k-tiles over the contraction dim
WNAMES = ("wq", "wk", "wv", "wo")


def _band_mask_np():
    # maskT[jt, jj, i] = 1 iff j <= i <= j+W, j = jt*128+jj  (layout [j, i])
    j = np.arange(C)[:, None]
    i = np.arange(C)[None, :]
    m = ((j <= i) & (i <= j + W)).astype(np.float16)
    return np.ascontiguousarray(m.reshape(2, 128, C))


def _emit(ctx, tc, io, n_chunks):
    nc = tc.nc
    x_d, w_d, mask_d, y_d = io

    singles = ctx.enter_context(tc.tile_pool(name="singles", bufs=1))
    xpool = ctx.enter_context(tc.tile_pool(name="xpool", bufs=2))
    qkpool = ctx.enter_context(tc.tile_pool(name="qkpool", bufs=2))
    vpool = ctx.enter_context(tc.tile_pool(name="vpool", bufs=2))
    opool = ctx.enter_context(tc.tile_pool(name="opool", bufs=2))
    otpool = ctx.enter_context(tc.tile_pool(name="otpool", bufs=2))
    ypool = ctx.enter_context(tc.tile_pool(name="ypool", bufs=2))
    dnpool = ctx.enter_context(tc.tile_pool(name="dnpool", bufs=4))

    # PSUM: 8 banks of [128, 2KB]. ps256 serves QK projection, scores, and
    # transposes; ps512 serves V/O projections; psbig the PV accumulators.
    # 4 + 2 + 2 = 8 banks exactly.
    ps256 = ctx.enter_context(tc.tile_pool(name="ps256", bufs=4, space="PSUM"))
    ps512 = ctx.enter_context(tc.tile_pool(name="ps512", bufs=2, space="PSUM"))
    psbig = ctx.enter_context(tc.tile_pool(name="psbig", bufs=2, space="PSUM"))

    # --- constants / weights resident in SBUF (fp16) ---
    # Weights/mask load on the GPSIMD (SWDGE) queues so the per-chunk x/y
    # traffic on the HWDGE queues is not stuck behind the weights at start.
    w_sb = {}
    for wn in WNAMES:
        kts = []
        for kt in range(KT):
            t = singles.tile([128, D], F16, tag=f"{wn}{kt}", name=f"{wn}{kt}")
            nc.gpsimd.dma_start(
                out=t, in_=w_d[wn].ap()[kt * 128:(kt + 1) * 128, :])
            kts.append(t)
        w_sb[wn] = kts
    mask_sb = singles.tile([128, 2, C], F16, tag="mask")
    nc.gpsimd.dma_start(out=mask_sb, in_=mask_d.ap().rearrange("jt p i -> p jt i"))
    ident = singles.tile([128, 128], F32, tag="ident")
    make_identity(nc, ident)

    # Persistent probs tiles, one per (head-pair, j-tile), reused every chunk.
    # The [192:256) tail of the jt=0 tiles is only ever zero (PV reads it as
    # padding) -- zero it once here instead of every chunk.
    p_sb = {}
    for hm in range(8):
        for jt in range(2):
            t = singles.tile([128, 2, C], F16, tag=f"p{hm}_{jt}",
                             name=f"p{hm}_{jt}")
            if jt == 0:
                nc.vector.memset(t[:, :, 192:C], 0.0)
            p_sb[(hm, jt)] = t

    state = {}

    def load_x(c):
        x_sb = xpool.tile([128, KT, C], F16, tag="x", name=f"x{c}")
        tok0 = (c % CPC) * C
        nc.sync.dma_start(
            out=x_sb,
            in_=x_d.ap()[:, tok0:tok0 + C].rearrange("(kt p) t -> p kt t", p=128))
        state[c] = {"x": x_sb}

    def stage_a(c):
        """Projections for chunk c: qT/kT/v. 12 yield blocks."""
        if c + 1 < n_chunks:
            load_x(c + 1)
        st = state[c]
        x_sb = st["x"]
        # qT, kT -> fp16 [dout-par, m, tok]; two m-tiles share a PSUM bank
        for wn, cp in (("wq", nc.scalar.copy), ("wk", nc.vector.tensor_copy)):
            qkt = qkpool.tile([128, KT, C], F16, tag=wn + "T", name=f"{wn}T{c}")
            for m in range(KT):
                ps = ps256.tile([128, C], F32, tag="ps256")
                msl = slice(m * 128, (m + 1) * 128)
                for kt in range(KT):
                    nc.tensor.matmul(ps, w_sb[wn][kt][:, msl], x_sb[:, kt, :],
                                     start=(kt == 0), stop=(kt == KT - 1))
                cp(out=qkt[:, m, :], in_=ps)
                if m % 2:
                    yield
            st[wn] = qkt
        # v: [j-par, jt, head, 65] fp16 with ones column for the denominator
        v_sb = vpool.tile([128, 2, H, DH + 1], F16, tag="v", name=f"v{c}")
        nc.vector.memset(v_sb[:, :, :, DH:], 1.0)
        for jt in range(2):
            jsl = slice(jt * 128, (jt + 1) * 128)
            for nn in range(2):
                ps = ps512.tile([128, 512], F32, tag="ps512")
                nsl = slice(nn * 512, (nn + 1) * 512)
                for kt in range(KT):
                    nc.tensor.matmul(ps, x_sb[:, kt, jsl],
                                     w_sb["wv"][kt][:, nsl],
                                     start=(kt == 0), stop=(kt == KT - 1))
                nc.vector.tensor_copy(
                    out=v_sb[:, jt, nn * 8:(nn + 1) * 8, :DH],
                    in_=ps.rearrange("p (h d) -> p h d", h=8))
                yield
        st["v"] = v_sb

    def stage_b(c):
        """Attention + output projection for chunk c. 7 yield blocks."""
        st = state[c]
        qT, kT, v_sb = st["wq"], st["wk"], st["v"]
        tok0 = (c % CPC) * C
        oN = [opool.tile([128, D], F32, tag="oN", name=f"oN{c}_{i}")
              for i in range(2)]
        for qt in range(4):
            o_ps = [psbig.tile([128, 4, DH + 1], F32, tag="obig",
                               name=f"o_ps{c}_{qt}_{i}") for i in range(2)]
            # head pairs (2*hm, 2*hm+1) sit on partitions 0-63 / 64-127; their
            # K=64 score matmuls use disjoint PE row groups and are emitted
            # interleaved so they run concurrently in the array. All four
            # (pr, jt) score+softmax groups are emitted before any PV matmul
            # so the exp->mask->PV latency hides under other PE work.
            lo64, hi64 = slice(0, 64), slice(64, 128)
            # Band structure (j <= i <= j+64): j-tile0 only ever feeds
            # i in [0,192), j-tile1 only i in [128,256). Compute scores,
            # exp and mask only on those column bands; p's [192,256)
            # region for j-tile0 is zero-filled once so PV can run full-M.
            bsl = (slice(0, 192), slice(128, C))
            for pr in range(2):
                hm = qt * 2 + pr
                for jt in range(2):
                    jsl = slice(jt * 128, (jt + 1) * 128)
                    isl = bsl[jt]
                    s0 = ps256.tile([128, C], F32, tag="ps256",
                                    name=f"s0_{c}_{hm}_{jt}")
                    s1 = ps256.tile([128, C], F32, tag="ps256",
                                    name=f"s1_{c}_{hm}_{jt}")
                    nc.tensor.matmul(s0[:, isl], kT[lo64, hm, jsl],
                                     qT[lo64, hm, isl], start=True, stop=True)
                    nc.tensor.matmul(s1[:, isl], kT[hi64, hm, jsl],
                                     qT[hi64, hm, isl], start=True, stop=True)
                    p2 = p_sb[(hm, jt)]
                    for hp, s_ps in ((0, s0), (1, s1)):
                        nc.scalar.activation(
                            out=p2[:, hp, isl], in_=s_ps[:, isl],
                            func=mybir.ActivationFunctionType.Exp, scale=0.125)
                    mask_bc = bass.AP(
                        tensor=mask_sb.tensor,
                        offset=mask_sb[:, jt, isl].offset,
                        ap=[mask_sb.ap[0], [0, 2], mask_sb[:, jt, isl].ap[-1]])
                    nc.vector.tensor_mul(p2[:, :, isl], p2[:, :, isl], mask_bc)
                yield
            # PV (+den via ones column), all full-M matmuls:
            # i-tile0 <- j-tile0 only; i-tile1 <- j-tile0 (cols [128,192)
            # live, rest zero-filled) accumulated with j-tile1.
            for pr in range(2):
                hm = qt * 2 + pr
                for hp, h in ((0, 2 * hm), (1, 2 * hm + 1)):
                    hq = h - qt * 4
                    nc.tensor.matmul(
                        o_ps[0][:, hq, :],
                        p_sb[(hm, 0)][:, hp, 0:128],
                        v_sb[:, 0, h, :],
                        start=True, stop=True)
                    nc.tensor.matmul(
                        o_ps[1][:, hq, :],
                        p_sb[(hm, 0)][:, hp, 128:C],
                        v_sb[:, 0, h, :],
                        start=True, stop=False)
                    nc.tensor.matmul(
                        o_ps[1][:, hq, :],
                        p_sb[(hm, 1)][:, hp, 128:C],
                        v_sb[:, 1, h, :],
                        start=False, stop=True)
            # normalize this quarter: oN = oU * (1/den), fused in PSUM copy
            for it in range(2):
                denr = dnpool.tile([128, 4], F32, tag="denr")
                nc.vector.reciprocal(out=denr, in_=o_ps[it][:, :, DH])
                denr_bc = bass.AP(
                    tensor=denr.tensor, offset=denr.offset,
                    ap=[denr.ap[0], denr.ap[1], [0, DH]])
                nc.vector.tensor_mul(
                    oN[it][:, qt * 256:(qt + 1) * 256]
                    .rearrange("p (h d) -> p h d", h=4),
                    o_ps[it][:, :, :DH],
                    denr_bc)
            yield
        # transpose oN -> oT [dout-par, dt, i]; two d-tiles (4 transposes)
        # share a PSUM bank and one fp16-converting copy
        oT = otpool.tile([128, KT, C], F16, tag="oT", name=f"oT{c}")
        for dt in range(KT):
            ps = ps256.tile([128, C], F32, tag="ps256", name=f"tp_{c}_{dt}")
            for it in range(2):
                nc.tensor.transpose(ps[:, it * 128:(it + 1) * 128],
                                    oN[it][:, dt * 128:(dt + 1) * 128], ident)
            nc.scalar.copy(out=oT[:, dt, :], in_=ps)
        yield
        # y projection + store (fp16)
        for it in range(2):
            isl = slice(it * 128, (it + 1) * 128)
            for nn in range(2):
                ps = ps512.tile([128, 512], F32, tag="ps512")
                nsl = slice(nn * 512, (nn + 1) * 512)
                for dt in range(KT):
                    nc.tensor.matmul(ps, oT[:, dt, isl],
                                     w_sb["wo"][dt][:, nsl],
                                     start=(dt == 0), stop=(dt == KT - 1))
                y_sb = ypool.tile([128, 512], F16, tag="y")
                nc.vector.tensor_copy(out=y_sb, in_=ps)
                nc.sync.dma_start(
                    out=y_d.ap()[tok0 + it * 128:tok0 + (it + 1) * 128, nsl],
                    in_=y_sb)
            yield

    def drain(g, n=10 ** 9):
        if g is None:
            return True
        for _ in range(n):
            try:
                next(g)
            except StopIteration:
                return True
        return False

    # prologue: chunk 0 projections un-interleaved (nothing to overlap with)
    load_x(0)
    drain(stage_a(0))
    for c in range(n_chunks):
        ga = stage_a(c + 1) if c + 1 < n_chunks else None
        gb = stage_b(c)
        a_done, b_done = ga is None, False
        while not (a_done and b_done):
            if not a_done:
                a_done = drain(ga, 1)
            if not b_done:
                b_done = drain(gb, 1)


def build(n_chunks=CPC, n_cores=NCORES):
    nc = bacc.Bacc("TRN2", target_bir_lowering=False, debug=False,
                   num_devices=n_cores)
    x_d = nc.dram_tensor("xt", [D, TPC], F16, kind="ExternalInput")
    w_d = {}
    for wn in WNAMES:
        w_d[wn] = nc.dram_tensor(wn, [D, D], F16, kind="ExternalInput")
    mask_d = nc.dram_tensor("maskt", [2, 128, C], F16, kind="ExternalInput")
    y_d = nc.dram_tensor("y", [TPC, D], F16, kind="ExternalOutput")
    io = (x_d, w_d, mask_d, y_d)
    with tile.TileContext(nc) as tc, ExitStack() as ctx:
        _emit(ctx, tc, io, n_chunks)
    nc.compile()
    return nc


def make_in_maps(x, Wq, Wk, Wv, Wo):
    xc = np.asarray(x, np.float32).reshape(NCHUNKS_TOTAL, C, D)
    mask = _band_mask_np()
    wmap = {wn: np.asarray(w, np.float32).astype(np.float16)
            for wn, w in zip(WNAMES, (Wq, Wk, Wv, Wo))}
    in_maps = []
    for s in range(NCORES):
        shard = xc[s * CPC:(s + 1) * CPC].reshape(TPC, D)
        xT = np.ascontiguousarray(shard.T.astype(np.float16))
        in_maps.append({"xt": xT, "maskt": mask, **wmap})
    return in_maps


_NC_CACHE = {}


def kernel(x, Wq, Wk, Wv, Wo):
    if "nc" not in _NC_CACHE:
        _NC_CACHE["nc"] = build()
    nc = _NC_CACHE["nc"]
    in_maps = make_in_maps(x, Wq, Wk, Wv, Wo)
    res = run_bass_kernel_spmd(nc, in_maps, core_ids=list(range(NCORES)))
    out = np.concatenate([res.results[s]["y"] for s in range(NCORES)], axis=0)
    return out.reshape(B, S, D).astype(np.float32)


# revision 22
# speedup vs baseline: 1.1442x; 1.1442x over previous
"""Chunked local attention (B=4, S=8192, D=1024, H=16, Dh=64, C=256, W=64)
on 8 Trainium2 NeuronCores.

Sharding: data-parallel over the 128 (batch x chunk) units -> 16 chunks/core.
The host passes each core its x shard pre-transposed in fp16 ([D, 16*256]);
weights are replicated fp16.

All matmuls run single-pass fp16 (1 PE cycle/row, fp32 PSUM accumulate).
End-to-end max-rel error ~5e-4 against the fp32 reference -- well inside
the 2e-2 gate -- at 3x less PE work than the previous bf16 hi/lo 3-pass
scheme (which targeted 2e-5).

The emission is software-pipelined: chunk c's attention phase (scores,
softmax, PV, transpose, Wo) is interleaved instruction-by-instruction with
chunk c+1's Q/K/V projection matmuls. Without this the attention phase's
small matmuls leave enough PE idle gaps that the HAM clock gate re-throttles
the PE to 1.2 GHz (measured: transpose/score matmuls at exactly the cold
rate); the dense interleaved stream keeps the PE at 2.4 GHz throughout.

ACT/DVE instruction pressure is kept low by batching pairs: both heads of a
score head-pair live in one [128,2,C] PSUM bank (one exp + one mask-mul per
pair), Q/K projection PSUM is written two m-tiles per bank (one copy per
pair), and transposes are copied out two d-tiles at a time.

Per-core dataflow:
  qT/kT = Wq^T/Wk^T @ xT     (lhsT=W native, rhs=xT)        [dout, tok]
  v     = xT^T @ Wv          (lhsT=xT, rhs=Wv)              [tok, dout]
  sT_h  = kT_h^T-x-qT_h      (lhsT=kT_h, rhs=qT_h)          [j, i] per head
  p_h   = exp(0.125*sT_h) * bandmask  (fp16)                [j, i]
  oU|den= p_h^T @ [v_h|1]    (lhsT=p_h, rhs=v_aug)          [i, 65] per head
          (ones column on v gives the softmax denominator for free;
           out-of-band p region is zero-filled so all PV matmuls are
           full-partition with simple start/stop accumulation)
  oN    = oU * (1/den)  per-head bcast, fused in PSUM copy
  oT    = PE-transpose(oN) fp32, fp16 conversion in the PSUM copy
  y     = oT^T @ Wo          (lhsT=oT, rhs=Wo)              [tok, d2] fp16
"""

from contextlib import ExitStack

import numpy as np

import concourse.bass as bass
import concourse.mybir as mybir
import concourse.tile as tile
from concourse import bacc
from concourse.bass_utils import run_bass_kernel_spmd
from concourse.masks import make_identity

B, S, D = 4, 8192, 1024
H, DH, C, W = 16, 64, 256, 64
NCORES = 8
NCHUNKS_TOTAL = B * (S // C)      # 128
CPC = NCHUNKS_TOTAL // NCORES     # 16 chunks per core
TPC = CPC * C                     # 4096 tokens per core
F32 = mybir.dt.float32
F16 = mybir.dt.float16
KT = D // 128                     # 8 k-tiles over the contraction dim
WNAMES = ("wq", "wk", "wv", "wo")


def _band_mask_np():
    # maskT[jt, jj, i] = 1 iff j <= i <= j+W, j = jt*128+jj  (layout [j, i])
    j = np.arange(C)[:, None]
    i = np.arange(C)[None, :]
    m = ((j <= i) & (i <= j + W)).astype(np.float16)
    return np.ascontiguousarray(m.reshape(2, 128, C))


def _emit(ctx, tc, io, n_chunks):
    nc = tc.nc
    x_d, w_d, mask_d, y_d = io

    singles = ctx.enter_context(tc.tile_pool(name="singles", bufs=1))
    xpool = ctx.enter_context(tc.tile_pool(name="xpool", bufs=2))
    qkpool = ctx.enter_context(tc.tile_pool(name="qkpool", bufs=2))
    vpool = ctx.enter_context(tc.tile_pool(name="vpool", bufs=2))
    opool = ctx.enter_context(tc.tile_pool(name="opool", bufs=2))
    otpool = ctx.enter_context(tc.tile_pool(name="otpool", bufs=2))
    ypool = ctx.enter_context(tc.tile_pool(name="ypool", bufs=2))
    dnpool = ctx.enter_context(tc.tile_pool(name="dnpool", bufs=4))

    # PSUM: 8 banks of [128, 2KB]. ps256 serves QK projection, scores, and
    # transposes; ps512 serves V/O projections; psbig the PV accumulators.
    # 4 + 2 + 2 = 8 banks exactly.
    ps256 = ctx.enter_context(tc.tile_pool(name="ps256", bufs=4, space="PSUM"))
    ps512 = ctx.enter_context(tc.tile_pool(name="ps512", bufs=2, space="PSUM"))
    psbig = ctx.enter_context(tc.tile_pool(name="psbig", bufs=2, space="PSUM"))

    # --- constants / weights resident in SBUF (fp16) ---
    # Weights/mask load on the GPSIMD (SWDGE) queues so the per-chunk x/y
    # traffic on the HWDGE queues is not stuck behind the weights at start.
    w_sb = {}
    for wn in WNAMES:
        kts = []
        for kt in range(KT):
            t = singles.tile([128, D], F16, tag=f"{wn}{kt}", name=f"{wn}{kt}")
            nc.gpsimd.dma_start(
                out=t, in_=w_d[wn].ap()[kt * 128:(kt + 1) * 128, :])
            kts.append(t)
        w_sb[wn] = kts
    mask_sb = singles.tile([128, 2, C], F16, tag="mask")
    nc.gpsimd.dma_start(out=mask_sb, in_=mask_d.ap().rearrange("jt p i -> p jt i"))
    ident = singles.tile([128, 128], F32, tag="ident")
    make_identity(nc, ident)

    # Persistent probs tiles, one per (head-pair, j-tile), reused every chunk.
    # The [192:256) tail of the jt=0 tiles is only ever zero (PV reads it as
    # padding) -- zero it once here instead of every chunk.
    p_sb = {}
    for hm in range(8):
        for jt in range(2):
            t = singles.tile([128, 2, C], F16, tag=f"p{hm}_{jt}",
                             name=f"p{hm}_{jt}")
            if jt == 0:
                nc.vector.memset(t[:, :, 192:C], 0.0)
            p_sb[(hm, jt)] = t

    state = {}

    def load_x(c):
        x_sb = xpool.tile([128, KT, C], F16, tag="x", name=f"x{c}")
        tok0 = (c % CPC) * C
        nc.sync.dma_start(
            out=x_sb,
            in_=x_d.ap()[:, tok0:tok0 + C].rearrange("(kt p) t -> p kt t", p=128))
        state[c] = {"x": x_sb}

    def stage_a(c):
        """Projections for chunk c: qT/kT/v. 12 yield blocks."""
        if c + 1 < n_chunks:
            load_x(c + 1)
        st = state[c]
        x_sb = st["x"]
        # qT, kT -> fp16 [dout-par, m, tok]; two m-tiles share a PSUM bank
        for wn, cp in (("wq", nc.scalar.copy), ("wk", nc.vector.tensor_copy)):
            qkt = qkpool.tile([128, KT, C], F16, tag=wn + "T", name=f"{wn}T{c}")
            for m in range(KT):
                ps = ps256.tile([128, C], F32, tag="ps256")
                msl = slice(m * 128, (m + 1) * 128)
                for kt in range(KT):
                    nc.tensor.matmul(ps, w_sb[wn][kt][:, msl], x_sb[:, kt, :],
                                     start=(kt == 0), stop=(kt == KT - 1))
                cp(out=qkt[:, m, :], in_=ps)
                if m % 2:
                    yield
            st[wn] = qkt
        # v: [j-par, jt, head, 65] fp16 with ones column for the denominator
        v_sb = vpool.tile([128, 2, H, DH + 1], F16, tag="v", name=f"v{c}")
        nc.vector.memset(v_sb[:, :, :, DH:], 1.0)
        for jt in range(2):
            jsl = slice(jt * 128, (jt + 1) * 128)
            for nn in range(2):
                ps = ps512.tile([128, 512], F32, tag="ps512")
                nsl = slice(nn * 512, (nn + 1) * 512)
                for kt in range(KT):
                    nc.tensor.matmul(ps, x_sb[:, kt, jsl],
                                     w_sb["wv"][kt][:, nsl],
                                     start=(kt == 0), stop=(kt == KT - 1))
                nc.vector.tensor_copy(
                    out=v_sb[:, jt, nn * 8:(nn + 1) * 8, :DH],
                    in_=ps.rearrange("p (h d) -> p h d", h=8))
                yield
        st["v"] = v_sb

    def stage_b(c):
        """Attention + output projection for chunk c. 7 yield blocks."""
        st = state[c]
        qT, kT, v_sb = st["wq"], st["wk"], st["v"]
        tok0 = (c % CPC) * C
        oN = [opool.tile([128, D], F32, tag="oN", name=f"oN{c}_{i}")
              for i in range(2)]
        for qt in range(4):
            o_ps = [psbig.tile([128, 4, DH + 1], F32, tag="obig",
                               name=f"o_ps{c}_{qt}_{i}") for i in range(2)]
            # head pairs (2*hm, 2*hm+1) sit on partitions 0-63 / 64-127; their
            # K=64 score matmuls use disjoint PE row groups and are emitted
            # interleaved so they run concurrently in the array.
            lo64, hi64 = slice(0, 64), slice(64, 128)
            # Band structure (j <= i <= j+64): j-tile0 only ever feeds
            # i in [0,192), j-tile1 only i in [128,256). Compute scores,
            # exp and mask only on those column bands; p's [192,256)
            # region for j-tile0 is zero-filled once so PV can run full-M.
            bsl = (slice(0, 192), slice(128, C))
            for pr in range(2):
                hm = qt * 2 + pr
                for jt in range(2):
                    jsl = slice(jt * 128, (jt + 1) * 128)
                    isl = bsl[jt]
                    s0 = ps256.tile([128, C], F32, tag="ps256",
                                    name=f"s0_{c}_{hm}_{jt}")
                    s1 = ps256.tile([128, C], F32, tag="ps256",
                                    name=f"s1_{c}_{hm}_{jt}")
                    nc.tensor.matmul(s0[:, isl], kT[lo64, hm, jsl],
                                     qT[lo64, hm, isl], start=True, stop=True)
                    nc.tensor.matmul(s1[:, isl], kT[hi64, hm, jsl],
                                     qT[hi64, hm, isl], start=True, stop=True)
                    p2 = p_sb[(hm, jt)]
                    for hp, s_ps in ((0, s0), (1, s1)):
                        nc.scalar.activation(
                            out=p2[:, hp, isl], in_=s_ps[:, isl],
                            func=mybir.ActivationFunctionType.Exp, scale=0.125)
                    mask_bc = bass.AP(
                        tensor=mask_sb.tensor,
                        offset=mask_sb[:, jt, isl].offset,
                        ap=[mask_sb.ap[0], [0, 2], mask_sb[:, jt, isl].ap[-1]])
                    nc.vector.tensor_mul(p2[:, :, isl], p2[:, :, isl], mask_bc)
                # PV (+den via ones column), all full-M matmuls:
                # i-tile0 <- j-tile0 only; i-tile1 <- j-tile0 (cols [128,192)
                # live, rest zero-filled) accumulated with j-tile1.
                for hp, h in ((0, 2 * hm), (1, 2 * hm + 1)):
                    hq = h - qt * 4
                    nc.tensor.matmul(
                        o_ps[0][:, hq, :],
                        p_sb[(hm, 0)][:, hp, 0:128],
                        v_sb[:, 0, h, :],
                        start=True, stop=True)
                    nc.tensor.matmul(
                        o_ps[1][:, hq, :],
                        p_sb[(hm, 0)][:, hp, 128:C],
                        v_sb[:, 0, h, :],
                        start=True, stop=False)
                    nc.tensor.matmul(
                        o_ps[1][:, hq, :],
                        p_sb[(hm, 1)][:, hp, 128:C],
                        v_sb[:, 1, h, :],
                        start=False, stop=True)
            # normalize this quarter: oN = oU * (1/den), fused in PSUM copy
            for it in range(2):
                denr = dnpool.tile([128, 4], F32, tag="denr")
                nc.vector.reciprocal(out=denr, in_=o_ps[it][:, :, DH])
                denr_bc = bass.AP(
                    tensor=denr.tensor, offset=denr.offset,
                    ap=[denr.ap[0], denr.ap[1], [0, DH]])
                nc.vector.tensor_mul(
                    oN[it][:, qt * 256:(qt + 1) * 256]
                    .rearrange("p (h d) -> p h d", h=4),
                    o_ps[it][:, :, :DH],
                    denr_bc)
            yield
        # transpose oN -> oT [dout-par, dt, i]; two d-tiles (4 transposes)
        # share a PSUM bank and one fp16-converting copy
        oT = otpool.tile([128, KT, C], F16, tag="oT", name=f"oT{c}")
        for dt in range(KT):
            ps = ps256.tile([128, C], F32, tag="ps256", name=f"tp_{c}_{dt}")
            for it in range(2):
                nc.tensor.transpose(ps[:, it * 128:(it + 1) * 128],
                                    oN[it][:, dt * 128:(dt + 1) * 128], ident)
            nc.scalar.copy(out=oT[:, dt, :], in_=ps)
        yield
        # y projection + store (fp16)
        for it in range(2):
            isl = slice(it * 128, (it + 1) * 128)
            for nn in range(2):
                ps = ps512.tile([128, 512], F32, tag="ps512")
                nsl = slice(nn * 512, (nn + 1) * 512)
                for dt in range(KT):
                    nc.tensor.matmul(ps, oT[:, dt, isl],
                                     w_sb["wo"][dt][:, nsl],
                                     start=(dt == 0), stop=(dt == KT - 1))
                y_sb = ypool.tile([128, 512], F16, tag="y")
                nc.vector.tensor_copy(out=y_sb, in_=ps)
                nc.sync.dma_start(
                    out=y_d.ap()[tok0 + it * 128:tok0 + (it + 1) * 128, nsl],
                    in_=y_sb)
            yield

    def drain(g, n=10 ** 9):
        if g is None:
            return True
        for _ in range(n):
            try:
                next(g)
            except StopIteration:
                return True
        return False

    # prologue: chunk 0 projections un-interleaved (nothing to overlap with)
    load_x(0)
    drain(stage_a(0))
    for c in range(n_chunks):
        ga = stage_a(c + 1) if c + 1 < n_chunks else None
        gb = stage_b(c)
        a_done, b_done = ga is None, False
        while not (a_done and b_done):
            if not a_done:
                a_done = drain(ga, 2)
            if not b_done:
                b_done = drain(gb, 1)


def build(n_chunks=CPC, n_cores=NCORES):
    nc = bacc.Bacc("TRN2", target_bir_lowering=False, debug=False,
                   num_devices=n_cores)
    x_d = nc.dram_tensor("xt", [D, TPC], F16, kind="ExternalInput")
    w_d = {}
    for wn in WNAMES:
        w_d[wn] = nc.dram_tensor(wn, [D, D], F16, kind="ExternalInput")
    mask_d = nc.dram_tensor("maskt", [2, 128, C], F16, kind="ExternalInput")
    y_d = nc.dram_tensor("y", [TPC, D], F16, kind="ExternalOutput")
    io = (x_d, w_d, mask_d, y_d)
    with tile.TileContext(nc) as tc, ExitStack() as ctx:
        _emit(ctx, tc, io, n_chunks)
    nc.compile()
    return nc


def make_in_maps(x, Wq, Wk, Wv, Wo):
    xc = np.asarray(x, np.float32).reshape(NCHUNKS_TOTAL, C, D)
    mask = _band_mask_np()
    wmap = {wn: np.asarray(w, np.float32).astype(np.float16)
            for wn, w in zip(WNAMES, (Wq, Wk, Wv, Wo))}
    in_maps = []
    for s in range(NCORES):
        shard = xc[s * CPC:(s + 1) * CPC].reshape(TPC, D)
        xT = np.ascontiguousarray(shard.T.astype(np.float16))
        in_maps.append({"xt": xT, "maskt": mask, **wmap})
    return in_maps


_NC_CACHE = {}


def kernel(x, Wq, Wk, Wv, Wo):
    if "nc" not in _NC_CACHE:
        _NC_CACHE["nc"] = build()
    nc = _NC_CACHE["nc"]
    in_maps = make_in_maps(x, Wq, Wk, Wv, Wo)
    res = run_bass_kernel_spmd(nc, in_maps, core_ids=list(range(NCORES)))
    out = np.concatenate([res.results[s]["y"] for s in range(NCORES)], axis=0)
    return out.reshape(B, S, D).astype(np.float32)


# revision 23
# speedup vs baseline: 1.1476x; 1.0029x over previous
"""Chunked local attention (B=4, S=8192, D=1024, H=16, Dh=64, C=256, W=64)
on 8 Trainium2 NeuronCores.

Sharding: data-parallel over the 128 (batch x chunk) units -> 16 chunks/core.
The host passes each core its x shard pre-transposed in fp16 ([D, 16*256]);
weights are replicated fp16.

All matmuls run single-pass fp16 (1 PE cycle/row, fp32 PSUM accumulate).
End-to-end max-rel error ~5e-4 against the fp32 reference -- well inside
the 2e-2 gate -- at 3x less PE work than the previous bf16 hi/lo 3-pass
scheme (which targeted 2e-5).

The emission is software-pipelined: chunk c's attention phase (scores,
softmax, PV, transpose, Wo) is interleaved instruction-by-instruction with
chunk c+1's Q/K/V projection matmuls. Without this the attention phase's
small matmuls leave enough PE idle gaps that the HAM clock gate re-throttles
the PE to 1.2 GHz (measured: transpose/score matmuls at exactly the cold
rate); the dense interleaved stream keeps the PE at 2.4 GHz throughout.

ACT/DVE instruction pressure is kept low by batching pairs: both heads of a
score head-pair live in one [128,2,C] PSUM bank (one exp + one mask-mul per
pair), Q/K projection PSUM is written two m-tiles per bank (one copy per
pair), and transposes are copied out two d-tiles at a time.

Per-core dataflow:
  qT/kT = Wq^T/Wk^T @ xT     (lhsT=W native, rhs=xT)        [dout, tok]
  v     = xT^T @ Wv          (lhsT=xT, rhs=Wv)              [tok, dout]
  sT_h  = kT_h^T-x-qT_h      (lhsT=kT_h, rhs=qT_h)          [j, i] per head
  p_h   = exp(0.125*sT_h) * bandmask  (fp16)                [j, i]
  oU|den= p_h^T @ [v_h|1]    (lhsT=p_h, rhs=v_aug)          [i, 65] per head
          (ones column on v gives the softmax denominator for free;
           out-of-band p region is zero-filled so all PV matmuls are
           full-partition with simple start/stop accumulation)
  oN    = oU * (1/den)  per-head bcast, fused in PSUM copy
  oT    = PE-transpose(oN) fp32, fp16 conversion in the PSUM copy
  y     = oT^T @ Wo          (lhsT=oT, rhs=Wo)              [tok, d2] fp16
"""

from contextlib import ExitStack

import numpy as np

import concourse.bass as bass
import concourse.mybir as mybir
import concourse.tile as tile
from concourse import bacc
from concourse.bass_utils import run_bass_kernel_spmd
from concourse.masks import make_identity

B, S, D = 4, 8192, 1024
H, DH, C, W = 16, 64, 256, 64
NCORES = 8
NCHUNKS_TOTAL = B * (S // C)      # 128
CPC = NCHUNKS_TOTAL // NCORES     # 16 chunks per core
TPC = CPC * C                     # 4096 tokens per core
F32 = mybir.dt.float32
F16 = mybir.dt.float16
KT = D // 128                     # 8 k-tiles over the contraction dim
WNAMES = ("wq", "wk", "wv", "wo")


def _band_mask_np():
    # maskT[jt, jj, i] = 1 iff j <= i <= j+W, j = jt*128+jj  (layout [j, i])
    j = np.arange(C)[:, None]
    i = np.arange(C)[None, :]
    m = ((j <= i) & (i <= j + W)).astype(np.float16)
    return np.ascontiguousarray(m.reshape(2, 128, C))


def _emit(ctx, tc, io, n_chunks):
    nc = tc.nc
    x_d, w_d, mask_d, y_d = io

    singles = ctx.enter_context(tc.tile_pool(name="singles", bufs=1))
    xpool = ctx.enter_context(tc.tile_pool(name="xpool", bufs=2))
    qkpool = ctx.enter_context(tc.tile_pool(name="qkpool", bufs=2))
    vpool = ctx.enter_context(tc.tile_pool(name="vpool", bufs=2))
    opool = ctx.enter_context(tc.tile_pool(name="opool", bufs=2))
    otpool = ctx.enter_context(tc.tile_pool(name="otpool", bufs=2))
    ypool = ctx.enter_context(tc.tile_pool(name="ypool", bufs=2))
    dnpool = ctx.enter_context(tc.tile_pool(name="dnpool", bufs=4))

    # PSUM: 8 banks of [128, 2KB]. ps256 serves QK projection, scores, and
    # transposes; ps512 serves V/O projections; psbig the PV accumulators.
    # 4 + 2 + 2 = 8 banks exactly.
    ps256 = ctx.enter_context(tc.tile_pool(name="ps256", bufs=4, space="PSUM"))
    ps512 = ctx.enter_context(tc.tile_pool(name="ps512", bufs=2, space="PSUM"))
    psbig = ctx.enter_context(tc.tile_pool(name="psbig", bufs=2, space="PSUM"))

    # --- constants / weights resident in SBUF (fp16) ---
    # Weights/mask load on the GPSIMD (SWDGE) queues so the per-chunk x/y
    # traffic on the HWDGE queues is not stuck behind the weights at start.
    w_sb = {}
    for wn in WNAMES:
        kts = []
        for kt in range(KT):
            t = singles.tile([128, D], F16, tag=f"{wn}{kt}", name=f"{wn}{kt}")
            nc.gpsimd.dma_start(
                out=t, in_=w_d[wn].ap()[kt * 128:(kt + 1) * 128, :])
            kts.append(t)
        w_sb[wn] = kts
    mask_sb = singles.tile([128, 2, C], F16, tag="mask")
    nc.gpsimd.dma_start(out=mask_sb, in_=mask_d.ap().rearrange("jt p i -> p jt i"))
    ident = singles.tile([128, 128], F32, tag="ident")
    make_identity(nc, ident)

    # Persistent probs tiles, one per (head-pair, j-tile), reused every chunk.
    # The [192:256) tail of the jt=0 tiles is only ever zero (PV reads it as
    # padding) -- zero it once here instead of every chunk.
    p_sb = {}
    for hm in range(8):
        for jt in range(2):
            t = singles.tile([128, 2, C], F16, tag=f"p{hm}_{jt}",
                             name=f"p{hm}_{jt}")
            if jt == 0:
                nc.vector.memset(t[:, :, 192:C], 0.0)
            p_sb[(hm, jt)] = t

    state = {}

    def load_x(c):
        x_sb = xpool.tile([128, KT, C], F16, tag="x", name=f"x{c}")
        tok0 = (c % CPC) * C
        nc.sync.dma_start(
            out=x_sb,
            in_=x_d.ap()[:, tok0:tok0 + C].rearrange("(kt p) t -> p kt t", p=128))
        state[c] = {"x": x_sb}

    def stage_a(c):
        """Projections for chunk c: qT/kT/v. 12 yield blocks."""
        if c + 1 < n_chunks:
            load_x(c + 1)
        st = state[c]
        x_sb = st["x"]
        # qT, kT -> fp16 [dout-par, m, tok]; two m-tiles share a PSUM bank
        for wn, cp in (("wq", nc.scalar.copy), ("wk", nc.vector.tensor_copy)):
            qkt = qkpool.tile([128, KT, C], F16, tag=wn + "T", name=f"{wn}T{c}")
            for m in range(KT):
                ps = ps256.tile([128, C], F32, tag="ps256")
                msl = slice(m * 128, (m + 1) * 128)
                for kt in range(KT):
                    nc.tensor.matmul(ps, w_sb[wn][kt][:, msl], x_sb[:, kt, :],
                                     start=(kt == 0), stop=(kt == KT - 1))
                cp(out=qkt[:, m, :], in_=ps)
                if m % 2:
                    yield
            st[wn] = qkt
        # v: [j-par, jt, head, 65] fp16 with ones column for the denominator
        v_sb = vpool.tile([128, 2, H, DH + 1], F16, tag="v", name=f"v{c}")
        nc.vector.memset(v_sb[:, :, :, DH:], 1.0)
        for jt in range(2):
            jsl = slice(jt * 128, (jt + 1) * 128)
            for nn in range(2):
                ps = ps512.tile([128, 512], F32, tag="ps512")
                nsl = slice(nn * 512, (nn + 1) * 512)
                for kt in range(KT):
                    nc.tensor.matmul(ps, x_sb[:, kt, jsl],
                                     w_sb["wv"][kt][:, nsl],
                                     start=(kt == 0), stop=(kt == KT - 1))
                nc.vector.tensor_copy(
                    out=v_sb[:, jt, nn * 8:(nn + 1) * 8, :DH],
                    in_=ps.rearrange("p (h d) -> p h d", h=8))
                yield
        st["v"] = v_sb

    def stage_b(c):
        """Attention + output projection for chunk c. 7 yield blocks."""
        st = state[c]
        qT, kT, v_sb = st["wq"], st["wk"], st["v"]
        tok0 = (c % CPC) * C
        oN = [opool.tile([128, D], F32, tag="oN", name=f"oN{c}_{i}")
              for i in range(2)]
        for qt in range(4):
            o_ps = [psbig.tile([128, 4, DH + 1], F32, tag="obig",
                               name=f"o_ps{c}_{qt}_{i}") for i in range(2)]
            # head pairs (2*hm, 2*hm+1) sit on partitions 0-63 / 64-127; their
            # K=64 score matmuls use disjoint PE row groups and are emitted
            # interleaved so they run concurrently in the array.
            lo64, hi64 = slice(0, 64), slice(64, 128)
            # Band structure (j <= i <= j+64): j-tile0 only ever feeds
            # i in [0,192), j-tile1 only i in [128,256). Compute scores,
            # exp and mask only on those column bands; p's [192,256)
            # region for j-tile0 is zero-filled once so PV can run full-M.
            bsl = (slice(0, 192), slice(128, C))
            for pr in range(2):
                hm = qt * 2 + pr
                for jt in range(2):
                    jsl = slice(jt * 128, (jt + 1) * 128)
                    isl = bsl[jt]
                    s0 = ps256.tile([128, C], F32, tag="ps256",
                                    name=f"s0_{c}_{hm}_{jt}")
                    s1 = ps256.tile([128, C], F32, tag="ps256",
                                    name=f"s1_{c}_{hm}_{jt}")
                    nc.tensor.matmul(s0[:, isl], kT[lo64, hm, jsl],
                                     qT[lo64, hm, isl], start=True, stop=True)
                    nc.tensor.matmul(s1[:, isl], kT[hi64, hm, jsl],
                                     qT[hi64, hm, isl], start=True, stop=True)
                    p2 = p_sb[(hm, jt)]
                    for hp, s_ps in ((0, s0), (1, s1)):
                        nc.scalar.activation(
                            out=p2[:, hp, isl], in_=s_ps[:, isl],
                            func=mybir.ActivationFunctionType.Exp, scale=0.125)
                        nc.vector.tensor_mul(p2[:, hp, isl], p2[:, hp, isl],
                                             mask_sb[:, jt, isl])
                # PV (+den via ones column), all full-M matmuls:
                # i-tile0 <- j-tile0 only; i-tile1 <- j-tile0 (cols [128,192)
                # live, rest zero-filled) accumulated with j-tile1.
                for hp, h in ((0, 2 * hm), (1, 2 * hm + 1)):
                    hq = h - qt * 4
                    nc.tensor.matmul(
                        o_ps[0][:, hq, :],
                        p_sb[(hm, 0)][:, hp, 0:128],
                        v_sb[:, 0, h, :],
                        start=True, stop=True)
                    nc.tensor.matmul(
                        o_ps[1][:, hq, :],
                        p_sb[(hm, 0)][:, hp, 128:C],
                        v_sb[:, 0, h, :],
                        start=True, stop=False)
                    nc.tensor.matmul(
                        o_ps[1][:, hq, :],
                        p_sb[(hm, 1)][:, hp, 128:C],
                        v_sb[:, 1, h, :],
                        start=False, stop=True)
            # normalize this quarter: oN = oU * (1/den), fused in PSUM copy
            for it in range(2):
                denr = dnpool.tile([128, 4], F32, tag="denr")
                nc.vector.reciprocal(out=denr, in_=o_ps[it][:, :, DH])
                denr_bc = bass.AP(
                    tensor=denr.tensor, offset=denr.offset,
                    ap=[denr.ap[0], denr.ap[1], [0, DH]])
                nc.vector.tensor_mul(
                    oN[it][:, qt * 256:(qt + 1) * 256]
                    .rearrange("p (h d) -> p h d", h=4),
                    o_ps[it][:, :, :DH],
                    denr_bc)
            yield
        # transpose oN -> oT [dout-par, dt, i]; two d-tiles (4 transposes)
        # share a PSUM bank and one fp16-converting copy
        oT = otpool.tile([128, KT, C], F16, tag="oT", name=f"oT{c}")
        for dt in range(KT):
            ps = ps256.tile([128, C], F32, tag="ps256", name=f"tp_{c}_{dt}")
            for it in range(2):
                nc.tensor.transpose(ps[:, it * 128:(it + 1) * 128],
                                    oN[it][:, dt * 128:(dt + 1) * 128], ident)
            nc.scalar.copy(out=oT[:, dt, :], in_=ps)
        yield
        # y projection + store (fp16)
        for it in range(2):
            isl = slice(it * 128, (it + 1) * 128)
            for nn in range(2):
                ps = ps512.tile([128, 512], F32, tag="ps512")
                nsl = slice(nn * 512, (nn + 1) * 512)
                for dt in range(KT):
                    nc.tensor.matmul(ps, oT[:, dt, isl],
                                     w_sb["wo"][dt][:, nsl],
                                     start=(dt == 0), stop=(dt == KT - 1))
                y_sb = ypool.tile([128, 512], F16, tag="y")
                nc.vector.tensor_copy(out=y_sb, in_=ps)
                nc.sync.dma_start(
                    out=y_d.ap()[tok0 + it * 128:tok0 + (it + 1) * 128, nsl],
                    in_=y_sb)
            yield

    def drain(g, n=10 ** 9):
        if g is None:
            return True
        for _ in range(n):
            try:
                next(g)
            except StopIteration:
                return True
        return False

    # prologue: chunk 0 projections un-interleaved (nothing to overlap with)
    load_x(0)
    drain(stage_a(0))
    for c in range(n_chunks):
        ga = stage_a(c + 1) if c + 1 < n_chunks else None
        gb = stage_b(c)
        a_done, b_done = ga is None, False
        while not (a_done and b_done):
            if not a_done:
                a_done = drain(ga, 2)
            if not b_done:
                b_done = drain(gb, 1)


def build(n_chunks=CPC, n_cores=NCORES):
    nc = bacc.Bacc("TRN2", target_bir_lowering=False, debug=False,
                   num_devices=n_cores)
    x_d = nc.dram_tensor("xt", [D, TPC], F16, kind="ExternalInput")
    w_d = {}
    for wn in WNAMES:
        w_d[wn] = nc.dram_tensor(wn, [D, D], F16, kind="ExternalInput")
    mask_d = nc.dram_tensor("maskt", [2, 128, C], F16, kind="ExternalInput")
    y_d = nc.dram_tensor("y", [TPC, D], F16, kind="ExternalOutput")
    io = (x_d, w_d, mask_d, y_d)
    with tile.TileContext(nc) as tc, ExitStack() as ctx:
        _emit(ctx, tc, io, n_chunks)
    nc.compile()
    return nc


def make_in_maps(x, Wq, Wk, Wv, Wo):
    xc = np.asarray(x, np.float32).reshape(NCHUNKS_TOTAL, C, D)
    mask = _band_mask_np()
    wmap = {wn: np.asarray(w, np.float32).astype(np.float16)
            for wn, w in zip(WNAMES, (Wq, Wk, Wv, Wo))}
    in_maps = []
    for s in range(NCORES):
        shard = xc[s * CPC:(s + 1) * CPC].reshape(TPC, D)
        xT = np.ascontiguousarray(shard.T.astype(np.float16))
        in_maps.append({"xt": xT, "maskt": mask, **wmap})
    return in_maps


_NC_CACHE = {}


def kernel(x, Wq, Wk, Wv, Wo):
    if "nc" not in _NC_CACHE:
        _NC_CACHE["nc"] = build()
    nc = _NC_CACHE["nc"]
    in_maps = make_in_maps(x, Wq, Wk, Wv, Wo)
    res = run_bass_kernel_spmd(nc, in_maps, core_ids=list(range(NCORES)))
    out = np.concatenate([res.results[s]["y"] for s in range(NCORES)], axis=0)
    return out.reshape(B, S, D).astype(np.float32)


# revision 29
# speedup vs baseline: 1.1522x; 1.0041x over previous
"""Chunked local attention (B=4, S=8192, D=1024, H=16, Dh=64, C=256, W=64)
on 8 Trainium2 NeuronCores.

Sharding: data-parallel over the 128 (batch x chunk) units -> 16 chunks/core.
The host passes each core its x shard pre-transposed in fp16 ([D, 16*256]);
weights are replicated fp16.

All matmuls run single-pass fp16 (1 PE cycle/row, fp32 PSUM accumulate).
End-to-end max-rel error ~5e-4 against the fp32 reference -- well inside
the 2e-2 gate -- at 3x less PE work than the previous bf16 hi/lo 3-pass
scheme (which targeted 2e-5).

The emission is software-pipelined: chunk c's attention phase (scores,
softmax, PV, transpose, Wo) is interleaved instruction-by-instruction with
chunk c+1's Q/K/V projection matmuls. Without this the attention phase's
small matmuls leave enough PE idle gaps that the HAM clock gate re-throttles
the PE to 1.2 GHz (measured: transpose/score matmuls at exactly the cold
rate); the dense interleaved stream keeps the PE at 2.4 GHz throughout.

ACT/DVE instruction pressure is kept low by batching pairs: both heads of a
score head-pair live in one [128,2,C] PSUM bank (one exp + one mask-mul per
pair), Q/K projection PSUM is written two m-tiles per bank (one copy per
pair), and transposes are copied out two d-tiles at a time.

Per-core dataflow:
  qT/kT = Wq^T/Wk^T @ xT     (lhsT=W native, rhs=xT)        [dout, tok]
  v     = xT^T @ Wv          (lhsT=xT, rhs=Wv)              [tok, dout]
  sT_h  = kT_h^T-x-qT_h      (lhsT=kT_h, rhs=qT_h)          [j, i] per head
  p_h   = exp(0.125*sT_h) * bandmask  (fp16)                [j, i]
  oU|den= p_h^T @ [v_h|1]    (lhsT=p_h, rhs=v_aug)          [i, 65] per head
          (ones column on v gives the softmax denominator for free;
           out-of-band p region is zero-filled so all PV matmuls are
           full-partition with simple start/stop accumulation)
  oN    = oU * (1/den)  per-head bcast, fused in PSUM copy
  oT    = PE-transpose(oN) fp32, fp16 conversion in the PSUM copy
  y     = oT^T @ Wo          (lhsT=oT, rhs=Wo)              [tok, d2] fp16
"""

from contextlib import ExitStack

import numpy as np

import concourse.bass as bass
import concourse.mybir as mybir
import concourse.tile as tile
from concourse import bacc
from concourse.bass_utils import run_bass_kernel_spmd
from concourse.masks import make_identity

B, S, D = 4, 8192, 1024
H, DH, C, W = 16, 64, 256, 64
NCORES = 8
NCHUNKS_TOTAL = B * (S // C)      # 128
CPC = NCHUNKS_TOTAL // NCORES     # 16 chunks per core
TPC = CPC * C                     # 4096 tokens per core
F32 = mybir.dt.float32
F16 = mybir.dt.float16
KT = D // 128                     # 8 k-tiles over the contraction dim
WNAMES = ("wq", "wk", "wv", "wo")


def _band_mask_np():
    # maskT[jt, jj, i] = 1 iff j <= i <= j+W, j = jt*128+jj  (layout [j, i])
    j = np.arange(C)[:, None]
    i = np.arange(C)[None, :]
    m = ((j <= i) & (i <= j + W)).astype(np.float16)
    return np.ascontiguousarray(m.reshape(2, 128, C))


def _emit(ctx, tc, io, n_chunks):
    nc = tc.nc
    x_d, w_d, mask_d, y_d = io

    singles = ctx.enter_context(tc.tile_pool(name="singles", bufs=1))
    xpool = ctx.enter_context(tc.tile_pool(name="xpool", bufs=2))
    qkpool = ctx.enter_context(tc.tile_pool(name="qkpool", bufs=2))
    vpool = ctx.enter_context(tc.tile_pool(name="vpool", bufs=2))
    ppool = ctx.enter_context(tc.tile_pool(name="ppool", bufs=8))
    opool = ctx.enter_context(tc.tile_pool(name="opool", bufs=2))
    otpool = ctx.enter_context(tc.tile_pool(name="otpool", bufs=2))
    ypool = ctx.enter_context(tc.tile_pool(name="ypool", bufs=2))
    dnpool = ctx.enter_context(tc.tile_pool(name="dnpool", bufs=4))

    # PSUM: 8 banks of [128, 2KB]. ps256 serves QK projection, scores, and
    # transposes; ps512 serves V/O projections; psbig the PV accumulators.
    # 4 + 2 + 2 = 8 banks exactly.
    ps256 = ctx.enter_context(tc.tile_pool(name="ps256", bufs=4, space="PSUM"))
    ps512 = ctx.enter_context(tc.tile_pool(name="ps512", bufs=2, space="PSUM"))
    psbig = ctx.enter_context(tc.tile_pool(name="psbig", bufs=2, space="PSUM"))

    # --- constants / weights resident in SBUF (fp16) ---
    # Weights/mask load on the GPSIMD (SWDGE) queues so the per-chunk x/y
    # traffic on the HWDGE queues is not stuck behind the weights at start.
    w_sb = {}
    for wn in WNAMES:
        kts = []
        for kt in range(KT):
            t = singles.tile([128, D], F16, tag=f"{wn}{kt}", name=f"{wn}{kt}")
            nc.gpsimd.dma_start(
                out=t, in_=w_d[wn].ap()[kt * 128:(kt + 1) * 128, :])
            kts.append(t)
        w_sb[wn] = kts
    mask_sb = singles.tile([128, 2, C], F16, tag="mask")
    nc.gpsimd.dma_start(out=mask_sb, in_=mask_d.ap().rearrange("jt p i -> p jt i"))
    ident = singles.tile([128, 128], F32, tag="ident")
    make_identity(nc, ident)

    state = {}

    def load_x(c):
        x_sb = xpool.tile([128, KT, C], F16, tag="x", name=f"x{c}")
        tok0 = (c % CPC) * C
        nc.sync.dma_start(
            out=x_sb,
            in_=x_d.ap()[:, tok0:tok0 + C].rearrange("(kt p) t -> p kt t", p=128))
        state[c] = {"x": x_sb}

    def stage_a(c):
        """Projections for chunk c: qT/kT/v. 12 yield blocks."""
        if c + 1 < n_chunks:
            load_x(c + 1)
        st = state[c]
        x_sb = st["x"]
        # qT, kT -> fp16 [dout-par, m, tok]; two m-tiles share a PSUM bank
        for wn, cp in (("wq", nc.scalar.copy), ("wk", nc.vector.tensor_copy)):
            qkt = qkpool.tile([128, KT, C], F16, tag=wn + "T", name=f"{wn}T{c}")
            for m in range(KT):
                ps = ps256.tile([128, C], F32, tag="ps256")
                msl = slice(m * 128, (m + 1) * 128)
                for kt in range(KT):
                    nc.tensor.matmul(ps, w_sb[wn][kt][:, msl], x_sb[:, kt, :],
                                     start=(kt == 0), stop=(kt == KT - 1))
                cp(out=qkt[:, m, :], in_=ps)
                if m % 2:
                    yield
            st[wn] = qkt
        # v: [j-par, jt, head, 65] fp16 with ones column for the denominator
        v_sb = vpool.tile([128, 2, H, DH + 1], F16, tag="v", name=f"v{c}")
        nc.vector.memset(v_sb[:, :, :, DH:], 1.0)
        for jt in range(2):
            jsl = slice(jt * 128, (jt + 1) * 128)
            for nn in range(2):
                ps = ps512.tile([128, 512], F32, tag="ps512")
                nsl = slice(nn * 512, (nn + 1) * 512)
                for kt in range(KT):
                    nc.tensor.matmul(ps, x_sb[:, kt, jsl],
                                     w_sb["wv"][kt][:, nsl],
                                     start=(kt == 0), stop=(kt == KT - 1))
                nc.vector.tensor_copy(
                    out=v_sb[:, jt, nn * 8:(nn + 1) * 8, :DH],
                    in_=ps.rearrange("p (h d) -> p h d", h=8))
                yield
        st["v"] = v_sb

    def stage_b(c):
        """Attention + output projection for chunk c. 7 yield blocks."""
        st = state[c]
        qT, kT, v_sb = st["wq"], st["wk"], st["v"]
        tok0 = (c % CPC) * C
        oN = [opool.tile([128, D], F32, tag="oN", name=f"oN{c}_{i}")
              for i in range(2)]
        for qt in range(4):
            o_ps = [psbig.tile([128, 4, DH + 1], F32, tag="obig",
                               name=f"o_ps{c}_{qt}_{i}") for i in range(2)]
            # head pairs (2*hm, 2*hm+1) sit on partitions 0-63 / 64-127; their
            # K=64 score matmuls use disjoint PE row groups and are emitted
            # interleaved so they run concurrently in the array.
            lo64, hi64 = slice(0, 64), slice(64, 128)
            # Band structure (j <= i <= j+64): j-tile0 only ever feeds
            # i in [0,192), j-tile1 only i in [128,256). Compute scores,
            # exp and mask only on those column bands; p's [192,256)
            # region for j-tile0 is zero-filled once so PV can run full-M.
            bsl = (slice(0, 192), slice(128, C))
            for pr in range(2):
                hm = qt * 2 + pr
                probs = {}   # jt -> [128, 2(head), C] fp16 tile
                for jt in range(2):
                    jsl = slice(jt * 128, (jt + 1) * 128)
                    isl = bsl[jt]
                    s0 = ps256.tile([128, C], F32, tag="ps256",
                                    name=f"s0_{c}_{hm}_{jt}")
                    s1 = ps256.tile([128, C], F32, tag="ps256",
                                    name=f"s1_{c}_{hm}_{jt}")
                    nc.tensor.matmul(s0[:, isl], kT[lo64, hm, jsl],
                                     qT[lo64, hm, isl], start=True, stop=True)
                    nc.tensor.matmul(s1[:, isl], kT[hi64, hm, jsl],
                                     qT[hi64, hm, isl], start=True, stop=True)
                    p2 = ppool.tile([128, 2, C], F16, tag="probs",
                                    name=f"p_{c}_{hm}_{jt}")
                    probs[jt] = p2
                    for hp, s_ps in ((0, s0), (1, s1)):
                        if jt == 0:
                            nc.vector.memset(p2[:, hp, 192:C], 0.0)
                        nc.scalar.activation(
                            out=p2[:, hp, isl], in_=s_ps[:, isl],
                            func=mybir.ActivationFunctionType.Exp, scale=0.125)
                        nc.vector.tensor_mul(p2[:, hp, isl], p2[:, hp, isl],
                                             mask_sb[:, jt, isl])
                # PV (+den via ones column), all full-M matmuls:
                # i-tile0 <- j-tile0 only; i-tile1 <- j-tile0 (cols [128,192)
                # live, rest zero-filled) accumulated with j-tile1.
                for hp, h in ((0, 2 * hm), (1, 2 * hm + 1)):
                    hq = h - qt * 4
                    nc.tensor.matmul(
                        o_ps[0][:, hq, :],
                        probs[0][:, hp, 0:128],
                        v_sb[:, 0, h, :],
                        start=True, stop=True)
                    nc.tensor.matmul(
                        o_ps[1][:, hq, :],
                        probs[0][:, hp, 128:C],
                        v_sb[:, 0, h, :],
                        start=True, stop=False)
                    nc.tensor.matmul(
                        o_ps[1][:, hq, :],
                        probs[1][:, hp, 128:C],
                        v_sb[:, 1, h, :],
                        start=False, stop=True)
            # normalize this quarter: oN = oU * (1/den), fused in PSUM copy
            for it in range(2):
                denr = dnpool.tile([128, 4], F32, tag="denr")
                nc.vector.reciprocal(out=denr, in_=o_ps[it][:, :, DH])
                denr_bc = bass.AP(
                    tensor=denr.tensor, offset=denr.offset,
                    ap=[denr.ap[0], denr.ap[1], [0, DH]])
                nc.vector.tensor_mul(
                    oN[it][:, qt * 256:(qt + 1) * 256]
                    .rearrange("p (h d) -> p h d", h=4),
                    o_ps[it][:, :, :DH],
                    denr_bc)
            yield
        # transpose oN -> oT [dout-par, dt, i]; two d-tiles (4 transposes)
        # share a PSUM bank and one fp16-converting copy
        oT = otpool.tile([128, KT, C], F16, tag="oT", name=f"oT{c}")
        for dt in range(KT):
            ps = ps256.tile([128, C], F32, tag="ps256", name=f"tp_{c}_{dt}")
            for it in range(2):
                nc.tensor.transpose(ps[:, it * 128:(it + 1) * 128],
                                    oN[it][:, dt * 128:(dt + 1) * 128], ident)
            nc.scalar.copy(out=oT[:, dt, :], in_=ps)
        yield
        # y projection + store (fp16)
        for it in range(2):
            isl = slice(it * 128, (it + 1) * 128)
            for nn in range(2):
                ps = ps512.tile([128, 512], F32, tag="ps512")
                nsl = slice(nn * 512, (nn + 1) * 512)
                for dt in range(KT):
                    nc.tensor.matmul(ps, oT[:, dt, isl],
                                     w_sb["wo"][dt][:, nsl],
                                     start=(dt == 0), stop=(dt == KT - 1))
                y_sb = ypool.tile([128, 512], F16, tag="y")
                nc.scalar.copy(out=y_sb, in_=ps)
                nc.sync.dma_start(
                    out=y_d.ap()[tok0 + it * 128:tok0 + (it + 1) * 128, nsl],
                    in_=y_sb)
            yield

    def drain(g, n=10 ** 9):
        if g is None:
            return True
        for _ in range(n):
            try:
                next(g)
            except StopIteration:
                return True
        return False

    # prologue: chunk 0 projections un-interleaved (nothing to overlap with)
    load_x(0)
    drain(stage_a(0))
    for c in range(n_chunks):
        ga = stage_a(c + 1) if c + 1 < n_chunks else None
        gb = stage_b(c)
        a_done, b_done = ga is None, False
        while not (a_done and b_done):
            if not a_done:
                a_done = drain(ga, 2)
            if not b_done:
                b_done = drain(gb, 1)


def build(n_chunks=CPC, n_cores=NCORES):
    nc = bacc.Bacc("TRN2", target_bir_lowering=False, debug=False,
                   num_devices=n_cores)
    x_d = nc.dram_tensor("xt", [D, TPC], F16, kind="ExternalInput")
    w_d = {}
    for wn in WNAMES:
        w_d[wn] = nc.dram_tensor(wn, [D, D], F16, kind="ExternalInput")
    mask_d = nc.dram_tensor("maskt", [2, 128, C], F16, kind="ExternalInput")
    y_d = nc.dram_tensor("y", [TPC, D], F16, kind="ExternalOutput")
    io = (x_d, w_d, mask_d, y_d)
    with tile.TileContext(nc) as tc, ExitStack() as ctx:
        _emit(ctx, tc, io, n_chunks)
    nc.compile()
    return nc


def make_in_maps(x, Wq, Wk, Wv, Wo):
    xc = np.asarray(x, np.float32).reshape(NCHUNKS_TOTAL, C, D)
    mask = _band_mask_np()
    wmap = {wn: np.asarray(w, np.float32).astype(np.float16)
            for wn, w in zip(WNAMES, (Wq, Wk, Wv, Wo))}
    in_maps = []
    for s in range(NCORES):
        shard = xc[s * CPC:(s + 1) * CPC].reshape(TPC, D)
        xT = np.ascontiguousarray(shard.T.astype(np.float16))
        in_maps.append({"xt": xT, "maskt": mask, **wmap})
    return in_maps


_NC_CACHE = {}


def kernel(x, Wq, Wk, Wv, Wo):
    if "nc" not in _NC_CACHE:
        _NC_CACHE["nc"] = build()
    nc = _NC_CACHE["nc"]
    in_maps = make_in_maps(x, Wq, Wk, Wv, Wo)
    res = run_bass_kernel_spmd(nc, in_maps, core_ids=list(range(NCORES)))
    out = np.concatenate([res.results[s]["y"] for s in range(NCORES)], axis=0)
    return out.reshape(B, S, D).astype(np.float32)
